# revision 3
# baseline (speedup 1.0000x reference)
"""PointAttentionEncoder1D Trainium2 kernel (8-core data-parallel over points).

Math (per point p, K=16 neighbors):
  q,ck,cv = MLP_{0,1,2}(center); ok,ov = MLP_{3,4}(other_k); pos_k = MLP_5(c-o_k)
  slot0: pos_0 = MLP_5(0) (const)
  fea_weight = WP(q - k_all + pos); out = sum_k softmax_c(fea_weight) * (v_all + pos)

Kernel formulation (all matmuls channels-on-partitions, columns = points):
  u45/u35 = leaky(stage-A) hidden pairs, vv = ov+pos, t = WP-layer1(q-ok+pos),
  e = exp(WP-layer2 + bp2), S_k = sum_c e, invS = exp(-ln(S)),
  out = sum_k (e*vv)*bcast(invS)  via PE selector matmuls.
"""
import os
import sys

sys.path.insert(0, "/opt/trn_rl_repo")
import numpy as np
from contextlib import ExitStack

import concourse.bass as bass
from concourse import bacc
import concourse.tile as tile
from concourse import mybir
from concourse.bass_utils import run_bass_kernel_spmd

F32 = mybir.dt.float32
AF = mybir.ActivationFunctionType
ALU = mybir.AluOpType

P_TOTAL = 100000
NCORES = 8
PPC = 12500                      # points per core (real)
TILE = 512                       # points per tile iteration
NT = int(os.environ.get("KERNEL_NT", "25"))
PPAD = 25 * TILE                 # padded points per core (12800)
K = 16
SLOPE = 0.01


def _leaky(x):
    return np.where(x > 0, x, SLOPE * x)


def _make_consts(w1, b1, w2, b2, wp1, bp1, wp2, bp2):
    """Build all fused weight/selector/bias constants, packed into tensors.

    Returns dict name -> np.ndarray. All matmul lhsT tensors start at
    partition base 0.
    """
    f32 = np.float32
    pos0 = (b2[5] + _leaky(b1[5]) @ w2[5]).astype(f32)          # [32]

    # --- stage A: x0T rows [o(48); c(3); 1] = 52.  A1 [52, 1024] (8 chunks of 128)
    A1 = np.zeros((52, 1024), f32)
    for g in range(8):
        grp45 = g < 4
        for j in range(4):
            k = 4 * (g % 4) + j
            base = 128 * g + 32 * j
            i_first = 4 if grp45 else 3
            A1[3 * k:3 * k + 3, base:base + 16] = w1[i_first]
            A1[51, base:base + 16] = b1[i_first]
            A1[3 * k:3 * k + 3, base + 16:base + 32] = -w1[5]
            A1[48:51, base + 16:base + 32] = w1[5]
            A1[51, base + 16:base + 32] = b1[5]
    Auc = np.zeros((52, 48), f32)
    for i in range(3):
        Auc[48:51, 16 * i:16 * i + 16] = w1[i]
        Auc[51, 16 * i:16 * i + 16] = b1[i]

    # --- vv = ov + pos:   u45 tile rows [u4_k(16); u5_k(16)] x4k
    L_vv = np.zeros((128, 128), f32)
    for j in range(4):
        L_vv[32 * j:32 * j + 16, 32 * j:32 * j + 32] = w2[4]
        L_vv[32 * j + 16:32 * j + 32, 32 * j:32 * j + 32] = w2[5]
    bias_vv = np.tile(b2[4] + b2[5], 4).reshape(128, 1).astype(f32)

    # --- t = (q - ok + pos) @ wp1 + bp1, per k-pair.  contraction = [u3_a;u5_a;u3_b;u5_b]
    W3p = w2[3] @ wp1
    W5p = w2[5] @ wp1
    W0p = w2[0] @ wp1
    W1p = w2[1] @ wp1
    L_t = np.zeros((64, 128), f32)
    for s in range(2):
        L_t[32 * s:32 * s + 16, 64 * s:64 * s + 64] = -W3p
        L_t[32 * s + 16:32 * s + 32, 64 * s:64 * s + 64] = W5p
    L_tc = np.zeros((48, 128), f32)          # uc rows [u0;u1;u2] -> q@wp1 into both halves
    L_tc[0:16, 0:64] = W0p
    L_tc[0:16, 64:128] = W0p
    bias_t = np.tile((b2[0] - b2[3] + b2[5]) @ wp1 + bp1, 2).reshape(128, 1).astype(f32)

    # --- WP layer2 (e pre-act), per 4k group from two tt pair-tiles
    L_w2a = np.zeros((128, 128), f32)
    for s in range(2):
        L_w2a[64 * s:64 * s + 64, 32 * s:32 * s + 32] = wp2
    L_w2b = np.roll(L_w2a, 64, axis=1)
    bias_e = np.tile(bp2, 4).reshape(128, 1).astype(f32)

    # --- S selector: out row (k or 16=center) = sum of that k's 32 channels
    L_S = np.zeros((128, 128), f32)          # col block g: [:, 32g:32g+32]
    for g in range(4):
        for j in range(4):
            L_S[32 * j:32 * j + 32, 32 * g + 4 * g + j] = 1.0
    L_S0 = np.zeros((32, 32), f32)
    L_S0[:, 16] = 1.0

    # --- invS broadcast selector: [17, 512], block g: [:, 128g:128g+128]
    L_B = np.zeros((17, 512), f32)
    for g in range(4):
        for j in range(4):
            L_B[4 * g + j, 128 * g + 32 * j:128 * g + 32 * j + 32] = 1.0
    L_B0 = np.zeros((17, 32), f32)
    L_B0[16, :] = 1.0

    # --- ksum selector
    L_K = np.tile(np.eye(32, dtype=f32), (4, 1))      # [128, 32]
    I32 = np.eye(32, dtype=f32)
    I128 = np.eye(128, dtype=f32)

    # --- slot 0
    L_t0 = np.zeros((48, 64), f32)
    L_t0[0:16] = W0p
    L_t0[16:32] = -W1p
    bias_t0 = ((b2[0] - b2[1] + pos0) @ wp1 + bp1).reshape(64, 1).astype(f32)
    L_t20 = wp2.astype(f32)                            # [64, 32]
    bias_e0 = bp2.reshape(32, 1).astype(f32)
    L_vv0 = np.zeros((48, 32), f32)
    L_vv0[32:48] = w2[2]
    bias_vv0 = (b2[2] + pos0).reshape(32, 1).astype(f32)

    # ---- pack into two const tensors:
    #  cstA [128, NA]: everything with up-to-128 partition rows, column-concatenated
    blocks = dict(A1=A1, Auc=Auc, L_vv=L_vv, L_t=L_t, L_tc=L_tc, L_w2a=L_w2a,
                  L_w2b=L_w2b, L_S=L_S, L_S0=L_S0, L_B=L_B, L_B0=L_B0, L_K=L_K,
                  I32=I32, I128=I128, L_t0=L_t0, L_t20=L_t20, L_vv0=L_vv0,
                  bias_vv=bias_vv, bias_t=bias_t, bias_e=bias_e,
                  bias_t0=bias_t0, bias_e0=bias_e0, bias_vv0=bias_vv0)
    offs = {}
    col = 0
    for name, arr in blocks.items():
        offs[name] = (col, arr.shape[0], arr.shape[1])
        col += arr.shape[1]
    cstA = np.zeros((128, col), f32)
    for name, arr in blocks.items():
        c0, p, w = offs[name]
        cstA[0:p, c0:c0 + w] = arr
    return cstA, offs


def _build(ncols_cst):
    nc = bacc.Bacc("TRN2", target_bir_lowering=False, debug=False, num_devices=NCORES)
    xin = nc.declare_dram_parameter("xin", [PPAD, 52], F32, isOutput=False)
    cst = nc.declare_dram_parameter("cst", [128, ncols_cst], F32, isOutput=False)
    outp = nc.declare_dram_parameter("outp", [PPAD, 32], F32, isOutput=True)

    CO = None  # set below per name

    with tile.TileContext(nc) as tc, ExitStack() as ctx:
        cpool = ctx.enter_context(tc.tile_pool(name="cpool", bufs=1))
        sb_x = ctx.enter_context(tc.tile_pool(name="sb_x", bufs=3))
        sb_xt = ctx.enter_context(tc.tile_pool(name="sb_xt", bufs=2))
        sb_u = ctx.enter_context(tc.tile_pool(name="sb_u", bufs=2))
        sb_mid = ctx.enter_context(tc.tile_pool(name="sb_mid", bufs=2))
        sb_out = ctx.enter_context(tc.tile_pool(name="sb_out", bufs=2))
        ps_big = ctx.enter_context(tc.tile_pool(name="ps_big", bufs=3, space="PSUM"))
        ps_acc = ctx.enter_context(tc.tile_pool(name="ps_acc", bufs=2, space="PSUM"))
        ps_sml = ctx.enter_context(tc.tile_pool(name="ps_sml", bufs=2, space="PSUM"))

        ct = cpool.tile([128, ncols_cst], F32)
        nc.sync.dma_start(out=ct[:], in_=cst[:])

        def C(name, offs=None):
            c0, p, w = _build.offs[name]
            return ct[0:p, c0:c0 + w]

        for it in range(NT):
            p0 = it * TILE
            # ---- load + transpose input: x0T [52, 512]
            x0T_ps = ps_big.tile([128, TILE], F32, tag="psA")
            for c in range(4):
                xr = sb_x.tile([128, 52], F32, tag="xraw")
                nc.sync.dma_start(out=xr[:], in_=xin[p0 + 128 * c: p0 + 128 * (c + 1), :])
                nc.tensor.transpose(x0T_ps[0:52, 128 * c:128 * (c + 1)], xr[:], C("I128"))
            x0T = sb_xt.tile([52, TILE], F32, tag="x0T")
            nc.vector.tensor_copy(x0T[:], x0T_ps[0:52, :])

            # ---- stage A + leaky evac
            u45 = []
            u35p = []   # 8 pair tiles [64, 512]
            for g in range(8):
                zps = ps_big.tile([128, TILE], F32, tag="psA")
                nc.tensor.matmul(zps[:, :], C("A1")[:, 128 * g:128 * (g + 1)], x0T[:],
                                 start=True, stop=True)
                if g < 4:
                    ut = sb_u.tile([128, TILE], F32, tag=f"u45_{g}")
                    nc.scalar.activation(ut[:], zps[:, :], AF.Lrelu, alpha=SLOPE)
                    u45.append(ut)
                else:
                    ua = sb_u.tile([64, TILE], F32, tag=f"u35a_{g}")
                    ub = sb_u.tile([64, TILE], F32, tag=f"u35b_{g}")
                    nc.scalar.activation(ua[:], zps[0:64, :], AF.Lrelu, alpha=SLOPE)
                    nc.scalar.activation(ub[:], zps[64:128, :], AF.Lrelu, alpha=SLOPE)
                    u35p.extend([ua, ub])
            ucps = ps_sml.tile([64, TILE], F32, tag="psS")
            nc.tensor.matmul(ucps[0:48, :], C("Auc"), x0T[:], start=True, stop=True)
            uc = sb_u.tile([48, TILE], F32, tag="uc")
            nc.scalar.activation(uc[:], ucps[0:48, :], AF.Lrelu, alpha=SLOPE)

            # ---- vv (+bias via DVE)
            vvs = []
            for g in range(4):
                vps = ps_big.tile([128, TILE], F32, tag="psA")
                nc.tensor.matmul(vps[:, :], C("L_vv"), u45[g][:], start=True, stop=True)
                vt = sb_mid.tile([128, TILE], F32, tag=f"vv_{g}")
                nc.vector.tensor_scalar(vt[:], vps[:, :], C("bias_vv"), None, ALU.add)
                vvs.append(vt)

            # ---- t (pair mms + center accum) + leaky evac
            tts = []
            for p in range(8):
                tps = ps_big.tile([128, TILE], F32, tag="psA")
                nc.tensor.matmul(tps[:, :], C("L_t"), u35p[p][:], start=True, stop=False)
                nc.tensor.matmul(tps[:, :], C("L_tc"), uc[:], start=False, stop=True)
                tt = sb_mid.tile([128, TILE], F32, tag=f"tt_{p}")
                nc.scalar.activation(tt[:], tps[:, :], AF.Lrelu, bias=C("bias_t"), alpha=SLOPE)
                tts.append(tt)

            # ---- WP layer2 + exp evac
            es = []
            for g in range(4):
                eps = ps_big.tile([128, TILE], F32, tag="psA")
                nc.tensor.matmul(eps[:, :], C("L_w2a"), tts[2 * g][:], start=True, stop=False)
                nc.tensor.matmul(eps[:, :], C("L_w2b"), tts[2 * g + 1][:], start=False, stop=True)
                et = sb_mid.tile([128, TILE], F32, tag=f"e_{g}")
                nc.scalar.activation(et[:], eps[:, :], AF.Exp, bias=C("bias_e"))
                es.append(et)

            # ---- slot 0
            t0ps = ps_sml.tile([64, TILE], F32, tag="psS")
            nc.tensor.matmul(t0ps[:, :], C("L_t0"), uc[:], start=True, stop=True)
            tt0 = sb_mid.tile([64, TILE], F32, tag="tt0")
            nc.scalar.activation(tt0[:], t0ps[:, :], AF.Lrelu, bias=C("bias_t0"), alpha=SLOPE)
            t20ps = ps_sml.tile([32, TILE], F32, tag="psS")
            nc.tensor.matmul(t20ps[:, :], C("L_t20"), tt0[:], start=True, stop=True)
            e0 = sb_mid.tile([32, TILE], F32, tag="e0")
            nc.scalar.activation(e0[:], t20ps[:, :], AF.Exp, bias=C("bias_e0"))
            v0ps = ps_sml.tile([32, TILE], F32, tag="psS")
            nc.tensor.matmul(v0ps[:, :], C("L_vv0"), uc[:], start=True, stop=True)
            vv0 = sb_mid.tile([32, TILE], F32, tag="vv0")
            nc.vector.tensor_scalar(vv0[:], v0ps[:, :], C("bias_vv0"), None, ALU.add)

            # ---- S + invS = exp(-ln(S))
            Sps = ps_acc.tile([32, TILE], F32, tag="psAcc")
            for g in range(4):
                nc.tensor.matmul(Sps[:, :], C("L_S")[:, 32 * g:32 * (g + 1)], es[g][:],
                                 start=(g == 0), stop=False)
            nc.tensor.matmul(Sps[:, :], C("L_S0"), e0[:], start=False, stop=True)
            lnS = sb_mid.tile([17, TILE], F32, tag="lnS")
            nc.scalar.activation(lnS[:], Sps[0:17, :], AF.Ln)
            invS = sb_mid.tile([17, TILE], F32, tag="invS")
            nc.scalar.activation(invS[:], lnS[:], AF.Exp, scale=-1.0)

            # ---- g = e*vv ; gw = g * bcast(invS) ; ksum
            acc = ps_acc.tile([32, TILE], F32, tag="psAcc")
            for g in range(4):
                gg = sb_mid.tile([128, TILE], F32, tag="gg")
                nc.vector.tensor_mul(gg[:], es[g][:], vvs[g][:])
                bps = ps_big.tile([128, TILE], F32, tag="psA")
                nc.tensor.matmul(bps[:, :], C("L_B")[:, 128 * g:128 * (g + 1)], invS[:],
                                 start=True, stop=True)
                gw = sb_mid.tile([128, TILE], F32, tag="gw")
                nc.vector.tensor_mul(gw[:], gg[:], bps[:, :])
                nc.tensor.matmul(acc[:, :], C("L_K"), gw[:], start=(g == 0), stop=False)
            g0 = sb_mid.tile([32, TILE], F32, tag="g0")
            nc.vector.tensor_mul(g0[:], e0[:], vv0[:])
            b0ps = ps_sml.tile([32, TILE], F32, tag="psS")
            nc.tensor.matmul(b0ps[:, :], C("L_B0"), invS[:], start=True, stop=True)
            gw0 = sb_mid.tile([32, TILE], F32, tag="gw0")
            nc.vector.tensor_mul(gw0[:], g0[:], b0ps[:, :])
            nc.tensor.matmul(acc[:, :], C("I32"), gw0[:], start=False, stop=True)

            # ---- transpose out + store
            accT = sb_out.tile([32, TILE], F32, tag="accT")
            nc.vector.tensor_copy(accT[:], acc[:, :])
            oT = ps_acc.tile([128, 128], F32, tag="psAcc")
            for c in range(4):
                nc.tensor.transpose(oT[:, 32 * c:32 * (c + 1)],
                                    accT[:, 128 * c:128 * (c + 1)], C("I32"))
            osb = sb_out.tile([128, 128], F32, tag="osb")
            nc.vector.tensor_copy(osb[:], oT[:])
            nc.sync.dma_start(
                out=outp[p0:p0 + TILE, :].rearrange("(c l) m -> l c m", c=4),
                in_=osb[:].rearrange("l (c m) -> l c m", c=4))
    nc.compile()
    return nc


_CACHE = {}


def kernel(center, other, mlp_w1, mlp_b1, mlp_w2, mlp_b2, wp_w1, wp_b1, wp_w2, wp_b2):
    center = np.asarray(center, np.float32)
    other = np.asarray(other, np.float32)
    cstA, offs = _make_consts(np.asarray(mlp_w1, np.float32), np.asarray(mlp_b1, np.float32),
                              np.asarray(mlp_w2, np.float32), np.asarray(mlp_b2, np.float32),
                              np.asarray(wp_w1, np.float32), np.asarray(wp_b1, np.float32),
                              np.asarray(wp_w2, np.float32), np.asarray(wp_b2, np.float32))
    _build.offs = offs

    P = center.shape[0]
    # xin rows: [other(48) | center(3) | 1]
    xin = np.zeros((NCORES, PPAD, 52), np.float32)
    oth = other.reshape(P, 48)
    cen = center.reshape(P, 3)
    for i in range(NCORES):
        sl = slice(i * PPC, (i + 1) * PPC)
        xin[i, :PPC, 0:48] = oth[sl]
        xin[i, :PPC, 48:51] = cen[sl]
        xin[i, :PPC, 51] = 1.0
    xin[:, PPC:, 51] = 1.0   # padded rows: ones only (keeps exp/ln finite)

    key = ("nc", NT)
    if key not in _CACHE:
        _CACHE[key] = _build(cstA.shape[1])
    nc = _CACHE[key]

    in_maps = [{"xin": xin[i], "cst": cstA} for i in range(NCORES)]
    res = run_bass_kernel_spmd(nc, in_maps, list(range(NCORES))).results
    out = np.empty((P, 32), np.float32)
    for i in range(NCORES):
        out[i * PPC:(i + 1) * PPC] = res[i]["outp"][:PPC]
    return out
